# revision 1
# baseline (speedup 1.0000x reference)
"""Otsu-threshold binarize (nn_BinarizeLayer) on 8 Trainium2 NeuronCores, v2.

Pipeline (4 SPMD launches, data-parallel over batch):
  L1 stats   : exact f32 min (vector) / max (gpsimd TT-tree) + stride-16 and
               stride-64 bf16 subsamples (scalar engine strided copies).
  L2 coarse  : 63-edge cumulative counts of the stride-64 subsample at
               every-4th-bin granularity (vector imm-edge masks + grouped
               reduces, scalar Sign for the rest) -> j_hat1 (+-8 bins).
  L3 refine  : 16 single-bin edges around j_hat1 on the stride-16 subsample
               -> j_hat2 (+-1 bin, empirically).
  L4 fused   : one pass over x: exact window cleq at 5 consecutive integer
               edges (4 on vector-mask + PE colsum-accumulate into PSUM
               banks, 1 on scalar Sign), exact zsum/S partials (PE), AND the
               binarized output y = (x > T_spec) with T_spec = centers[j_hat2]
               speculated.  Host then computes the exact Otsu argmax from the
               window; if it confirms j_hat2, y is already correct; otherwise
               L4 is relaunched with the corrected threshold (same NEFF).

The z-shift trick makes every window/coarse edge a compile-time-immediate
integer: z_sh = rne(x*s + B) - 2^23 with the runtime window base folded into
the AP bias B, so comparisons run in the DVE's fast packed modes.
"""

import numpy as np
import ml_dtypes

import concourse.bass as bass
import concourse.mybir as mybir
from concourse.bass_utils import run_bass_kernel_spmd

F32 = mybir.dt.float32
BF16 = mybir.dt.bfloat16
ALU = mybir.AluOpType
AX = mybir.AxisListType
ACT = mybir.ActivationFunctionType

NCORES = 8
P = 128
FREE = 32768
CHUNK = 8192
NCHUNK = FREE // CHUNK
SHAPE = (16, 1024, 2048, 1)
NTOT = SHAPE[0] * SHAPE[1] * SHAPE[2] * SHAPE[3]

SUB16 = FREE // 16          # 2048 stride-16 subsample elems / partition
SUB64 = FREE // 64          # 512 stride-64 subsample elems / partition

NC1 = 63                    # coarse edges (every 4th bin)
NC1_V = 28                  # on vector
NC1_S = NC1 - NC1_V         # on scalar
NREF = 16                   # refine edges (single bin)
NREF_V = 11
NREF_S = NREF - NREF_V

WIN = 4                     # exact window cleq edges in L4

TWO23 = 8388608.0

TRACE = False
EXEC_TIMES_NS = []

_NC_CACHE = {}


def _run(nc, in_maps):
    res = run_bass_kernel_spmd(
        nc, in_maps, core_ids=list(range(NCORES)), trace=TRACE
    )
    if TRACE:
        EXEC_TIMES_NS.append(res.exec_time_ns)
    return res.results


# --------------------------------------------------------------------------
# L1: min/max + subsamples
# --------------------------------------------------------------------------

def _nc_stats():
    if "stats" in _NC_CACHE:
        return _NC_CACHE["stats"]
    nc = bass.Bass()
    x = nc.dram_tensor("x", [P, FREE], F32, kind="ExternalInput")
    mm = nc.dram_tensor("mm", [P, 2 * NCHUNK], F32, kind="ExternalOutput")
    sub64 = nc.dram_tensor("sub64", [P, SUB64], BF16, kind="ExternalOutput")
    with (
        nc.sbuf_tensor([P, 2, CHUNK], F32) as xt,
        nc.sbuf_tensor([P, 2 * NCHUNK], F32) as mms,
        nc.sbuf_tensor([P, SUB64], BF16) as s64t,
        nc.semaphore("dma_sem") as dma_sem,
        nc.semaphore("v_sem") as v_sem,
        nc.semaphore("s_sem") as s_sem,
        nc.Block() as block,
    ):
        @block.sync
        def _(sync):
            for i in range(NCHUNK):
                if i >= 2:
                    sync.wait_ge(v_sem, i - 1)
                    sync.wait_ge(s_sem, i - 1)
                sync.dma_start(
                    out=xt[:, i % 2, :], in_=x[:, i * CHUNK:(i + 1) * CHUNK]
                ).then_inc(dma_sem, 16)
            sync.wait_ge(v_sem, NCHUNK)
            sync.dma_start(out=mm[:, :], in_=mms[:, :]).then_inc(dma_sem, 16)
            sync.wait_ge(s_sem, NCHUNK)
            sync.dma_start(out=sub64[:, :], in_=s64t[:, :]).then_inc(dma_sem, 16)
            sync.wait_ge(dma_sem, 16 * (NCHUNK + 2))

        @block.vector
        def _(vector):
            for i in range(NCHUNK):
                vector.wait_ge(dma_sem, 16 * (i + 1))
                vector.tensor_reduce(
                    out=mms[:, 2 * i:2 * i + 1], in_=xt[:, i % 2, :],
                    axis=AX.X, op=ALU.min)
                vector.tensor_reduce(
                    out=mms[:, 2 * i + 1:2 * i + 2], in_=xt[:, i % 2, :],
                    axis=AX.X, op=ALU.max,
                ).then_inc(v_sem, 1)

        @block.scalar
        def _(scalar):
            for i in range(NCHUNK):
                scalar.wait_ge(dma_sem, 16 * (i + 1))
                xi = xt[:, i % 2, :]
                s64src = xi.rearrange("p (a s) -> p a s", s=64)
                n64 = CHUNK // 64
                scalar.activation(
                    out=s64t[:, i * n64:(i + 1) * n64], in_=s64src[:, :, 0],
                    func=ACT.Copy, bias=0.0, scale=1.0,
                ).then_inc(s_sem, 1)
    _NC_CACHE["stats"] = nc
    return nc


# --------------------------------------------------------------------------
# L2/L3: subsample histogram launches (coarse + refine share a template)
# --------------------------------------------------------------------------

def _nc_subhist(name, fd, nedges, nv):
    """Edges are is_le against immediate integers 0..nv-1 on vector; the
    remaining nedges-nv edges use scalar Sign with AP bias -(t+0.5) for
    t = nv..nedges-1.  Input values get the affine z-shift first:
    z = rne(xsub*s + B) - 2^23 (s, B are AP inputs)."""
    key = (name, fd, nedges, nv)
    if key in _NC_CACHE:
        return _NC_CACHE[key]
    ns = nedges - nv
    ngrp = (nv + 7) // 8
    nc = bass.Bass()
    xs = nc.dram_tensor("xs", [P, fd], BF16, kind="ExternalInput")
    par = nc.dram_tensor("par", [P, 2], F32, kind="ExternalInput")
    sbias = nc.dram_tensor("sbias", [P, max(ns, 1)], F32, kind="ExternalInput")
    acc = nc.dram_tensor("acc", [P, max(nv, 1)], F32, kind="ExternalOutput")
    sacc = nc.dram_tensor("sacc", [P, max(ns, 1)], F32, kind="ExternalOutput")
    with (
        nc.sbuf_tensor([P, fd], BF16) as xst,
        nc.sbuf_tensor([P, fd], F32) as wt,
        nc.sbuf_tensor([P, fd], BF16) as zt,
        nc.sbuf_tensor([P, 8, fd], BF16) as mt,
        nc.sbuf_tensor([P, fd], BF16) as dmp,
        nc.sbuf_tensor([P, 2], F32) as pt,
        nc.sbuf_tensor([P, max(ns, 1)], F32) as sbt,
        nc.sbuf_tensor([P, max(nv, 1)], F32) as at,
        nc.sbuf_tensor([P, max(ns, 1)], F32) as sat,
        nc.semaphore("dma_sem") as dma_sem,
        nc.semaphore("v_sem") as v_sem,
        nc.semaphore("s_sem") as s_sem,
        nc.Block() as block,
    ):
        @block.sync
        def _(sync):
            sync.dma_start(out=xst[:, :], in_=xs[:, :]).then_inc(dma_sem, 16)
            sync.dma_start(out=pt[:, :], in_=par[:, :]).then_inc(dma_sem, 16)
            sync.dma_start(out=sbt[:, :], in_=sbias[:, :]).then_inc(dma_sem, 16)
            sync.wait_ge(v_sem, 3)
            sync.dma_start(out=acc[:, :], in_=at[:, :]).then_inc(dma_sem, 16)
            if ns:
                sync.wait_ge(s_sem, 1)
            sync.dma_start(out=sacc[:, :], in_=sat[:, :]).then_inc(dma_sem, 16)
            sync.wait_ge(dma_sem, 16 * 5)

        @block.vector
        def _(vector):
            vector.wait_ge(dma_sem, 48)
            vector.tensor_scalar(
                out=wt[:, :], in0=xst[:, :], scalar1=pt[:, 0:1],
                scalar2=pt[:, 1:2], op0=ALU.mult, op1=ALU.add)
            vector.tensor_scalar(
                out=zt[:, :], in0=wt[:, :], scalar1=TWO23,
                scalar2=None, op0=ALU.subtract).then_inc(v_sem, 2)
            for t in range(nv):
                vector.tensor_scalar(
                    out=mt[:, t % 8, :], in0=zt[:, :],
                    scalar1=float(t), scalar2=None, op0=ALU.is_le)
                ins = vector.tensor_reduce(
                    out=at[:, t:t + 1], in_=mt[:, t % 8, :], axis=AX.X,
                    op=ALU.add)
                if t == nv - 1:
                    ins.then_inc(v_sem, 1)

        @block.scalar
        def _(scalar):
            scalar.wait_ge(v_sem, 2)
            for t in range(ns):
                ins = scalar.activation(
                    out=dmp[:, :], in_=zt[:, :], func=ACT.Sign,
                    bias=sbt[:, t:t + 1], scale=1.0,
                    accum_out=sat[:, t:t + 1])
                if t == ns - 1:
                    ins.then_inc(s_sem, 1)
    _NC_CACHE[key] = nc
    return nc


# --------------------------------------------------------------------------
# L4: fused exact window + speculative binarize
# --------------------------------------------------------------------------

def _nc_fused():
    if "fused" in _NC_CACHE:
        return _NC_CACHE["fused"]
    nc = bass.Bass()
    x = nc.dram_tensor("x", [P, FREE], F32, kind="ExternalInput")
    par = nc.dram_tensor("par", [P, 5], F32, kind="ExternalInput")
    # par: [s, B1(=2^23-0.5-mn*s-(j0-1)), T_spec, signx_bias(=-mn*s-(j0+3)), 0]
    y = nc.dram_tensor("y", [P, FREE], F32, kind="ExternalOutput")
    ps = nc.dram_tensor("ps", [1, 4 * 512], F32, kind="ExternalOutput")
    sacc = nc.dram_tensor("sacc", [P, 2 * NCHUNK], F32, kind="ExternalOutput")
    FP8 = mybir.dt.float8e4
    NPE = 4                    # psum slots: cleq t=0,1,2 + rneg
    from contextlib import ExitStack
    es = ExitStack()
    xt = es.enter_context(nc.sbuf_tensor([P, 2, CHUNK], F32))
    yt = es.enter_context(nc.sbuf_tensor([P, 2, CHUNK], F32))
    zt = es.enter_context(nc.sbuf_tensor([P, 2, CHUNK], BF16))
    mt = es.enter_context(nc.sbuf_tensor([P, 2, 4096], BF16))
    dmp = es.enter_context(nc.sbuf_tensor([P, CHUNK], BF16))
    pt = es.enter_context(nc.sbuf_tensor([P, 5], F32))
    ones = es.enter_context(nc.sbuf_tensor([P, 1], BF16))
    pst = es.enter_context(nc.sbuf_tensor([1, NPE * 512], F32))
    sat = es.enter_context(nc.sbuf_tensor([P, 2 * NCHUNK], F32))
    psum = es.enter_context(nc.psum_tensor([1, NPE * 512], F32))
    dma_sem = es.enter_context(nc.semaphore("dma_sem"))
    w_sem = es.enter_context(nc.semaphore("w_sem"))
    z_sem = es.enter_context(nc.semaphore("z_sem"))
    m_sem = es.enter_context(nc.semaphore("m_sem"))
    tm_sem = es.enter_context(nc.semaphore("tm_sem"))
    y_sem = es.enter_context(nc.semaphore("y_sem"))
    o_sem = es.enter_context(nc.semaphore("o_sem"))
    se_sem = es.enter_context(nc.semaphore("se_sem"))
    pc_sem = es.enter_context(nc.semaphore("pc_sem"))
    with nc.Block() as block:
        @block.sync
        def _(sync):
            sync.dma_start(out=pt[:, :], in_=par[:, :]).then_inc(dma_sem, 16)
            for i in range(NCHUNK):
                if i >= 2:
                    sync.wait_ge(w_sem, i - 1)         # scalar done with x
                    sync.wait_ge(y_sem, i - 1)         # vector y done with x
                sync.dma_start(
                    out=xt[:, i % 2, :], in_=x[:, i * CHUNK:(i + 1) * CHUNK]
                ).then_inc(dma_sem, 16)
            for i in range(NCHUNK):
                sync.wait_ge(y_sem, i + 1)
                sync.dma_start(
                    out=y[:, i * CHUNK:(i + 1) * CHUNK], in_=yt[:, i % 2, :]
                ).then_inc(o_sem, 16)
            sync.wait_ge(pc_sem, 1)
            sync.dma_start(out=ps[:, :], in_=pst[:, :]).then_inc(dma_sem, 16)
            sync.wait_ge(se_sem, NCHUNK)
            sync.dma_start(out=sacc[:, :], in_=sat[:, :]).then_inc(dma_sem, 16)
            sync.wait_ge(dma_sem, 16 * (NCHUNK + 3))
            sync.wait_ge(o_sem, 16 * NCHUNK)

        @block.scalar
        def _(scalar):
            for i in range(NCHUNK):
                scalar.wait_ge(dma_sem, 16 * (i + 2))
                if i >= 2:
                    scalar.wait_ge(o_sem, 16 * (i - 1))  # yt slot free again
                xi = xt[:, i % 2, :]
                # w = x*s + B1  (rne at 2^23 happens in the f32 add)
                scalar.activation(
                    out=yt[:, i % 2, :], in_=xi, func=ACT.Identity,
                    bias=pt[:, 1:2], scale=pt[:, 0:1],
                ).then_inc(w_sem, 1)
                # 4th window edge: cleq[j0+2] via Sign(z_sh - 3.5), z-based
                # for exact consistency with the PE mask edges
                scalar.wait_ge(z_sem, i + 1)
                scalar.activation(
                    out=dmp[:, :], in_=zt[:, i % 2, :], func=ACT.Sign,
                    bias=pt[:, 3:4], scale=1.0,
                    accum_out=sat[:, 2 * i:2 * i + 1])
                # S partial: sum z_sh
                scalar.activation(
                    out=dmp[:, :], in_=zt[:, i % 2, :], func=ACT.Identity,
                    bias=pt[:, 4:5], scale=1.0,
                    accum_out=sat[:, 2 * i + 1:2 * i + 2],
                ).then_inc(se_sem, 1)

        @block.vector
        def _(vector):
            vector.wait_ge(dma_sem, 16)
            vector.memset(ones[:, :], 1.0)
            for i in range(NCHUNK):
                xi = xt[:, i % 2, :]
                zi = zt[:, i % 2, :]
                # z_sh = w - 2^23 (bf16); w sits in yt until y overwrites it
                vector.wait_ge(w_sem, i + 1)
                if i >= 2:
                    vector.wait_ge(se_sem, i - 1)   # scalar done with z slot
                vector.tensor_scalar(
                    out=zi, in0=yt[:, i % 2, :], scalar1=TWO23,
                    scalar2=None, op0=ALU.subtract).then_inc(z_sem, 1)
                if i > 0:
                    # y = (x > T_spec), overwrites w in yt
                    vector.tensor_scalar(
                        out=yt[:, i % 2, :], in0=xi, scalar1=pt[:, 2:3],
                        scalar2=None, op0=ALU.is_gt).then_inc(y_sem, 1)
                # PE-fed half-masks: t=0,1,2 is_le + rneg(min 0)
                for t in range(NPE):
                    for h in range(2):
                        k = (i * NPE + t) * 2 + h
                        if k >= 2:
                            vector.wait_ge(tm_sem, k - 1)
                        zih = zi[:, h * 4096:(h + 1) * 4096]
                        if t < 3:
                            vector.tensor_scalar(
                                out=mt[:, k % 2, :], in0=zih,
                                scalar1=float(t), scalar2=None,
                                op0=ALU.is_le).then_inc(m_sem, 1)
                        else:
                            vector.tensor_scalar(
                                out=mt[:, k % 2, :], in0=zih, scalar1=0.0,
                                scalar2=None, op0=ALU.min).then_inc(m_sem, 1)
                if i == 0:
                    # chunk-0 y deferred until after its masks so the PE
                    # pipeline starts earlier
                    vector.tensor_scalar(
                        out=yt[:, 0, :], in0=xi, scalar1=pt[:, 2:3],
                        scalar2=None, op0=ALU.is_gt).then_inc(y_sem, 1)
            # copy psum slots to sbuf (host does exact f64 sums)
            vector.wait_ge(tm_sem, NCHUNK * NPE * 2)
            vector.tensor_copy(pst[:, :], psum[0:1, :]).then_inc(pc_sem, 1)

        @block.tensor
        def _(tensor):
            for i in range(NCHUNK):
                for t in range(NPE):
                    for h in range(2):
                        k = (i * NPE + t) * 2 + h
                        tensor.wait_ge(m_sem, k + 1)
                        mi = mt[:, k % 2, :]
                        for u in range(8):
                            ins = tensor.matmul(
                                psum[0:1, t * 512:(t + 1) * 512], ones[:, :],
                                mi[:, u * 512:(u + 1) * 512],
                                start=(i == 0 and h == 0 and u == 0),
                                stop=(i == NCHUNK - 1 and h == 1 and u == 7),
                                skip_group_check=True,
                            )
                            if u == 7:
                                ins.then_inc(tm_sem, 1)
    es.close()
    _NC_CACHE["fused"] = nc
    return nc


# --------------------------------------------------------------------------
# host-side otsu math (replicates reference.py numerics)
# --------------------------------------------------------------------------

def _edges_centers(mn, mx):
    """Replicate jnp.histogram's f32 bin edges + reference centers."""
    step = np.arange(256, dtype=np.float32) / np.float32(256.0)
    out = (mn * (np.float32(1.0) - step) + mx * step).astype(np.float32)
    edges = np.concatenate([out, np.asarray([mx], dtype=np.float32)])
    centers = (np.float32(0.5) * (edges[:-1] + edges[1:])).astype(np.float32)
    return edges, centers


def _merged_argmax(pts_j, pts_cleq, centers, N):
    """Otsu argmax estimate: linearly interpolate cleq over all 256 bins
    from estimates at integer bins pts_j, then run the reference V formula."""
    pj = np.asarray(pts_j, dtype=np.float64)
    pc = np.asarray(pts_cleq, dtype=np.float64)
    grid_j = np.concatenate([[-1.0], pj, [255.0]])
    grid_c = np.concatenate([[0.0], pc, [float(N)]])
    cleq_all = np.interp(np.arange(256, dtype=np.float64), grid_j, grid_c)
    cnt = np.diff(np.concatenate([[0.0], cleq_all]))
    c64 = centers.astype(np.float64)
    w1 = np.cumsum(cnt)
    w2 = np.cumsum(cnt[::-1])[::-1]
    cs = np.cumsum(cnt * c64)
    csr = np.cumsum((cnt * c64)[::-1])[::-1]
    with np.errstate(divide="ignore", invalid="ignore"):
        m1 = cs / np.maximum(w1, 1.0)
        m2 = csr / np.maximum(w2, 1.0)
        v = w1[:-1] * w2[1:] * (m1[:-1] - m2[1:]) ** 2
    return int(np.argmax(v))


# --------------------------------------------------------------------------
# main entry
# --------------------------------------------------------------------------

def kernel(inputs):
    x = np.asarray(inputs)
    assert x.shape == SHAPE, x.shape
    x = np.ascontiguousarray(x, dtype=np.float32)
    xs = x.reshape(NCORES, P, FREE)
    shards = [xs[c] for c in range(NCORES)]
    N = float(NTOT)

    # ---- L1: min/max + subsamples ----
    r = _run(_nc_stats(), [{"x": s} for s in shards])
    mm = np.stack([r[c]["mm"] for c in range(NCORES)]).reshape(
        NCORES, P, NCHUNK, 2)
    s64 = [r[c]["sub64"] for c in range(NCORES)]
    mn = np.float32(mm[..., 0].min())
    mx = np.float32(mm[..., 1].max())
    if not np.isfinite(mn) or not np.isfinite(mx) or mn == mx:
        return np.zeros(SHAPE, dtype=np.float32)

    scale = np.float32(256.0) / (mx - mn)
    edges, centers = _edges_centers(mn, mx)

    # ---- L2: coarse (64-bin) cumcounts on the stride-64 subsample ----
    # zc4 = rne(xsub*(s/4) - 0.5 - mn*(s/4)); edges at integers 0..62
    s4 = np.float32(scale) * np.float32(0.25)
    b4 = np.float32(TWO23) - np.float32(0.5) - np.float32(mn) * s4
    par = np.zeros((P, 2), dtype=np.float32)
    par[:, 0] = s4
    par[:, 1] = b4
    sb = -(np.arange(NC1_V, NC1, dtype=np.float32) + np.float32(0.5))
    sbias = np.tile(sb[None, :], (P, 1)).astype(np.float32)
    r = _run(_nc_subhist("c", SUB64, NC1, NC1_V),
             [{"xs": s64[c], "par": par, "sbias": sbias}
              for c in range(NCORES)])
    accv = np.stack([r[c]["acc"] for c in range(NCORES)]).astype(np.float64)
    accs = np.stack([r[c]["sacc"] for c in range(NCORES)]).astype(np.float64)
    n_s64 = float(NCORES * P * SUB64)
    cl_c = np.empty(NC1)
    cl_c[:NC1_V] = accv.sum(axis=(0, 1))[:NC1_V]
    cl_c[NC1_V:] = (n_s64 - accs.sum(axis=(0, 1))) / 2.0
    # coarse edge m covers z <= 4m+3  (zc4 <= m  <=>  z <= 4m+3)
    js_c = 4 * np.arange(1, NC1 + 1) - 1      # 3, 7, ..., 251
    j_hat = _merged_argmax(js_c, cl_c * 64.0, centers, N)

    # ---- L4: fused exact window + speculative binarize (with retry) ------
    centers64 = centers.astype(np.float64)
    A = centers64[0]
    B = (centers64[255] - centers64[0]) / 255.0

    y = None
    j_spec = j_hat
    for _attempt in range(24):
        j0 = int(np.clip(j_spec - 1, 1, 252))
        b1 = (np.float32(TWO23) - np.float32(0.5)
              - np.float32(mn) * scale - np.float32(j0 - 1))
        t_spec = np.float32(centers[j_spec])
        par = np.zeros((P, 5), dtype=np.float32)
        par[:, 0] = scale
        par[:, 1] = b1
        par[:, 2] = t_spec
        par[:, 3] = np.float32(-3.5)
        r = _run(_nc_fused(),
                 [{"x": shards[c], "par": par} for c in range(NCORES)])
        ps = np.stack([r[c]["ps"] for c in range(NCORES)]).astype(np.float64)
        sa = np.stack([r[c]["sacc"] for c in range(NCORES)]).astype(np.float64)
        slots = ps.reshape(NCORES, 4, 512).sum(axis=(0, 2))   # f64 exact
        sa = sa.reshape(NCORES, P, NCHUNK, 2).sum(axis=(0, 1, 2))
        cleq = {}
        for t in range(3):
            cleq[j0 - 1 + t] = slots[t]           # PE mask slots t=0,1,2
        cleq[j0 + 2] = (N - sa[0]) / 2.0          # scalar Sign-on-z edge
        zsum_below = slots[3] + (j0 - 1) * cleq[j0 - 1]    # sum z [z<=j0-1]
        S_z = sa[1] + (j0 - 1) * N                          # sum z
        S_c = A * N + B * S_z
        js = [j for j in range(j0, j0 + WIN - 1) if 0 <= j <= 254]
        vals = {}
        for j in js:
            w1 = cleq[j]
            w2 = N - w1
            cs = A * cleq[j0 - 1] + B * zsum_below
            for b in range(j0, j + 1):
                cs += (cleq[b] - cleq[b - 1]) * centers64[b]
            m1 = cs / max(w1, 1.0)
            m2 = (S_c - cs) / max(w2, 1.0)
            vals[j] = w1 * w2 * (m1 - m2) ** 2
        jbest = max(vals, key=lambda j: vals[j])
        lo, hi = js[0], js[-1]
        interior = (jbest > lo or lo == 0) and (jbest < hi or hi == 254)
        if interior:
            if jbest == j_spec:
                y = np.stack([r[c]["y"] for c in range(NCORES)])
                break
            j_spec = jbest       # threshold speculation missed; relaunch
        elif jbest == hi and hi != 254:
            j_spec = jbest + 2   # window missed high; jump past boundary
        elif jbest == lo and lo != 0:
            j_spec = jbest - 2
        else:
            j_spec = jbest
    assert y is not None
    return y.astype(np.float32).reshape(SHAPE)

